# revision 84
# baseline (speedup 1.0000x reference)
"""Trainium2 Bass kernel for nn_Attention_57672820850902.

Channel-attention block (XCA-style):
  kv = dwconv3x3(conv1x1(x)); k, v = split(kv)
  q  = conv3x3_full(conv1x1(y))
  q, k l2-normalized per channel row; attn = softmax(q @ k^T * temp) per head
  out = x - conv1x1(attn @ v)

Sharding: 8 cores = 4 batches x 2 spatial halves (64 rows + 1-row halo).
Cross-core traffic is two tiny pairwise AllReduces (norm sums, then the
per-head 32x32 attention logits).

Architecture (evolved from the v1 baseline; ~371us -> ~280-315us):
- The q 1x1 conv is FOLDED into the 3x3 on the host (W2 = q_dw @ q_w); the
  fused 3x3 runs as fp8-e4m3 DoubleRow matmuls (K=256/instr) over 14
  host-prebuilt shifted K-planes on the compact grid (7 tap-pair matmuls
  per M-half per 512-chunk). Per-output-channel weight scales cancel
  exactly in the l2 norm. The kv 1x1 conv is likewise fp8 DoubleRow.
- proj @ blockdiag(attn) is fused into one 192x192 matrix WcT on-device,
  so attn@v and the projection are a single matmul pass.
- Depthwise 3x3: k-lower on VectorE (row-quartered STT taps), k-upper/
  v-upper and v-lower via PE diagonal matmuls (v-lower rows 0:32 on DVE);
  the PE-diag halves run inside the phase-4/collective shadows.
- The AllReduce is SPLIT: norms first (issued before QK^T; hides the
  ~11us gpsimd collective-library cold start and lets fq/fk precompute
  off-path), then the logits on a warm library (~7us). Exp/Sqrt act
  tables are pre-warmed; the k-norm reciprocal runs after the partition
  broadcast (32 lanes, not 1).
- DMA: x/y ship as fp8 (chunk-major y planes: one contiguous descriptor
  per partition); the residual x preloads into SBUF aliasing the dead
  x8s/qt_full slots; residual input and output are bf16 (rel err ~5.7e-3
  vs the 2e-2 gate, dominated by those bf16 roundings).
- q/k spatial transposes for QK^T remain 512-wide xbar DMA transposes,
  serialized on the sync queue (concurrent xbar transposes from both
  HWDGE queues corrupt tiles). Measured dead ends, do not retry: PE
  transposes inside the QK^T stream (3x tried, ~60us WORSE: they break
  the attention accumulation-group flow), fp8 xbar transposes (API
  requires 2-byte dtypes), dummy collective warm-up, kt-interleave
  (wash), manual evac engine pinning (nc.any wins).
- Known remaining levers: orientation-swapped phase 4 under DoubleRow
  (would emit q pre-transposed, deleting the 40us qt chain; needs a
  partition-dim ssq reduction), and raw pairwise remote_dma in place of
  the second collective (~10us).
"""

import os
import numpy as np
import ml_dtypes

B, C, H, W, HEADS = 4, 192, 128, 128, 6
HC = C // HEADS                      # 32 channels per head
HP = H // 2                          # 64 rows per core
PH, PW = HP + 2, W + 2               # 66 x 130 padded shard
S_PAD = PH * PW                      # 8580
S_IN = HP * W                        # 8192
NCORES = 8
CA, CB = 128, 64                     # channel tile split of 192
CP = 256                             # K-padded channel count

bf16 = ml_dtypes.bfloat16

_cache = {}


def _pad_chunks():
    bounds = list(range(0, S_PAD, 512)) + [S_PAD]
    return list(zip(bounds[:-1], bounds[1:]))


def _build():
    import concourse.bass as bass
    import concourse.mybir as mybir
    import concourse.tile as tile
    from concourse import bacc

    dt = mybir.dt
    Alu = mybir.AluOpType
    Act = mybir.ActivationFunctionType

    nc = bacc.Bacc("TRN2", target_bir_lowering=False, debug=False,
                   num_devices=NCORES)

    # ---- per-core inputs ----
    # x for the kv conv as fp8 DoubleRow planes: plane0 = ch 0:128,
    # plane1 = ch 128:192 (+zero rows), both on the padded 66x130 grid
    # S_PAD rounded to 8592 (mult of 16) so the DoubleRow plane stride is legal
    x8_t = nc.dram_tensor("x_f8", [CA, 2, 8592], dt.float8e4, kind="ExternalInput")
    # y for the folded q 3x3 conv as 14 fp8 K-planes on the compact 64x128
    # output grid (shifts pre-baked on host): planes 0-8 = ch 0:128 taps 0-8;
    # planes 9+s = ch 128:192 [tap 2s rows 0:64 ; tap 2s+1 rows 64:128].
    # Chunk-major layout so each 512-col load is one contiguous descriptor
    # per partition.
    y8_t = nc.dram_tensor("y_f8", [CA, S_IN // 512, 14, 512], dt.float8e4,
                          kind="ExternalInput")
    x_ctr_t = nc.dram_tensor("x_ctr", [C, S_IN], dt.bfloat16, kind="ExternalInput")
    # ---- weights (same on all cores; K rows host-padded to 256) ----
    # kv_wT columns host-permuted to [k 0:128 | v 128:192 ; k 128:192 | v 0:128]
    kvw_t = nc.dram_tensor("kv_wT", [CA, 2, 2 * C], dt.float8e4, kind="ExternalInput")
    qdw_t = nc.dram_tensor("qdw_T", [CA, 14, C], dt.float8e4, kind="ExternalInput")
    # dw_all rows: [0:128]=k 0:128 | [128:192]=v 128:192 ; [192:256]=k 128:192 | [256:384]=v 0:128
    dw_t = nc.dram_tensor("dw_all", [384, 9], dt.float32, kind="ExternalInput")
    dwdiag_t = nc.dram_tensor("dw_diag", [9, 128, 128], dt.bfloat16, kind="ExternalInput")
    dwdiag2_t = nc.dram_tensor("dw_diag2", [9, 128, 128], dt.bfloat16, kind="ExternalInput")
    projw_t = nc.dram_tensor("proj_wT", [CP, C], dt.bfloat16, kind="ExternalInput")
    temp_t = nc.dram_tensor("temp", [HC, HEADS], dt.float32, kind="ExternalInput")
    out_t = nc.dram_tensor("out", [C, S_IN], dt.bfloat16, kind="ExternalOutput")

    PCH = _pad_chunks()
    taps = [(ky, kx) for ky in range(3) for kx in range(3)]
    NCH = S_IN // 512                # 16 inner chunks

    with tile.TileContext(nc) as tc:
        with tc.tile_pool(name="w", bufs=1) as wp, \
             tc.tile_pool(name="big", bufs=1) as bigp, \
             tc.tile_pool(name="io", bufs=3) as iop, \
             tc.tile_pool(name="io2", bufs=4) as iop2, \
             tc.tile_pool(name="tp", bufs=7) as tpp, \
             tc.tile_pool(name="ev", bufs=2) as evp, \
             tc.tile_pool(name="small", bufs=1) as smp, \
             tc.tile_pool(name="ps", bufs=6, space="PSUM") as psp, \
             tc.tile_pool(name="psattn", bufs=1, space="PSUM") as psattn, \
             tc.tile_pool(name="dram", bufs=1, space="DRAM") as dramp:

            # ---------- weights to SBUF ----------
            kvw8 = wp.tile([CA, 2, 2 * C], dt.float8e4)
            nc.sync.dma_start(kvw8[:], kvw_t.ap())
            qdw8 = wp.tile([CA, 14, C], dt.float8e4)
            nc.scalar.dma_start(qdw8[:], qdw_t.ap())
            dwk_a = wp.tile([CA, 9], dt.float32)
            dw_va = wp.tile([CA, 9], dt.float32)
            nc.sync.dma_start(dwk_a[:], dw_t.ap()[0:128])
            nc.sync.dma_start(dw_va[:], dw_t.ap()[256:384])
            dwdiag = wp.tile([128, 9, 128], dt.bfloat16)
            nc.scalar.dma_start(dwdiag[:], dwdiag_t.ap().rearrange("t k m -> k t m"))
            dwdiag2 = wp.tile([128, 9, 128], dt.bfloat16)
            nc.scalar.dma_start(dwdiag2[:], dwdiag2_t.ap().rearrange("t k m -> k t m"))
            projw_a = wp.tile([CA, C], dt.bfloat16)
            projw_b = wp.tile([CA, C], dt.bfloat16)
            nc.scalar.dma_start(projw_a[:], projw_t.ap()[0:CA])
            nc.scalar.dma_start(projw_b[:], projw_t.ap()[CA:CP])
            temp_s = wp.tile([HC, HEADS], dt.float32)
            nc.sync.dma_start(temp_s[:], temp_t.ap())
            warm = wp.tile([1, 1], dt.float32)
            nc.vector.memset(warm[:], 1.0)

            # ---------- persistent intermediates ----------
            k1a = bigp.tile([CA, PH, PW], dt.bfloat16, tag="k1a")
            # kv1b: rows 0:64 = v ch 128:192 ("v1b"), rows 64:128 = k ch 128:192 ("k1u")
            kv1b = bigp.tile([CA, PH, PW], dt.bfloat16, tag="kv1b")
            v1a = bigp.tile([CA, PH, PW], dt.bfloat16, tag="v1a")
            qt_full = bigp.tile([128, 64, C], dt.bfloat16, tag="qt_full")
            ka = bigp.tile([CA, S_IN], dt.bfloat16, tag="ka")
            # kvb_out: rows 0:64 = v ch 128:192 dw'd, rows 64:128 = k ch 128:192 dw'd
            kvb_out = bigp.tile([CA, S_IN], dt.bfloat16, tag="kvb_out")
            # va reuses the k1a slot (k1a dead after the k depthwise)
            va = bigp.tile([CA, S_IN], dt.bfloat16, tag="k1a")


            attn_pa = psattn.tile([CA, C], dt.float32, tag="attnA")
            attn_pb = psattn.tile([CB, C], dt.float32, tag="attnB")

            def flat(t):
                return t[:].rearrange("p h w -> p (h w)")

            # ---------- phase 1: kv1 = kv_w @ x (padded grid, fp8 DoubleRow) ----------
            #   psum0 = k[0:128]; psum1 = [v 128:192 ; k 128:192]; psum2 = v[0:128]
            # x8 loaded whole (17KB/partition), split across 4 queues so the
            # PE is never DMA-gated here.
            x8s = bigp.tile([CA, 2, 8592], dt.float8e4, tag="x8s")
            for qeng, lo, hi in ((nc.gpsimd, 0, 4296), (nc.sync, 4296, 8592)):
                qeng.dma_start(x8s[:, 0, lo:hi], x8_t.ap()[:, 0, lo:hi])
                qeng.dma_start(x8s[:, 1, lo:hi], x8_t.ap()[:, 1, lo:hi])
            for c0, c1 in PCH:
                n = c1 - c0
                p0 = psp.tile([CA, 512], dt.float32, tag="mm")
                p1 = psp.tile([CA, 512], dt.float32, tag="mm")
                for p, m0 in ((p0, 0), (p1, 128)):
                    nc.tensor.matmul(p[:, :n], kvw8[:, :, m0:m0 + 128],
                                     x8s[:, :, c0:c1],
                                     start=True, stop=True,
                                     perf_mode=mybir.MatmulPerfMode.DoubleRow)
                nc.any.tensor_copy(flat(k1a)[:, c0:c1], p0[:, :n])
                nc.any.tensor_copy(flat(kv1b)[:, c0:c1], p1[:, :n])

            # ---------- depthwise 3x3 taps (FMA on VectorE), row-chunked so
            # downstream per-chunk consumers (kt transposes, proj) unblock early
            def dw(dst, src, wsc, h0, nh):
                first = True
                for t, (ky, kx) in enumerate(taps):
                    shifted = src[:, h0 + ky:h0 + ky + nh, kx:kx + W]
                    d = dst[:].rearrange("p (h w) -> p h w", w=W)[:, h0:h0 + nh]
                    if first:
                        nc.vector.tensor_scalar(d, shifted, wsc[:, t:t + 1], None, Alu.mult)
                        first = False
                    else:
                        nc.vector.scalar_tensor_tensor(
                            d, shifted, wsc[:, t:t + 1], d, Alu.mult, Alu.add)

            # ---------- phase 2: depthwise on k-lower (VectorE, quartered);
            # k-upper/v-upper moved to PE diag inside the phase-4 loop ----------
            ssk_a = smp.tile([CA, 8], dt.float32)
            ssk_u = smp.tile([CA, NCH], dt.float32)   # rows 64:128 active
            for qtr in range(4):
                dw(ka, k1a, dwk_a, 16 * qtr, 16)
                for h in range(2):
                    i = 2 * qtr + h
                    sq = evp.tile([CA, 1024], dt.bfloat16, tag="sqscr")
                    nc.scalar.activation(sq[:], ka[:, 1024 * i:1024 * (i + 1)],
                                         Act.Square, accum_out=ssk_a[:, i:i + 1])


            # ---------- phase 4: q = folded 3x3 conv (fp8 DoubleRow, 7 K-pair
            # matmuls per M-half); evac -> transpose to qt_full ----------
            ssq_a = smp.tile([CA, 16], dt.float32)
            ssq_b = smp.tile([CB, 16], dt.float32)
            for i in range(NCH):
                s0 = 512 * i
                y8 = iop2.tile([CA, 14, 512], dt.float8e4, tag="y8")
                nc.scalar.dma_start(y8[:, 0:7], y8_t.ap()[:, i, 0:7])
                nc.gpsimd.dma_start(y8[:, 7:14], y8_t.ap()[:, i, 7:14])
                pqa = psp.tile([CA, 512], dt.float32, tag="mm")
                pqb = psp.tile([CB, 512], dt.float32, tag="mm")
                for p, m0, mw in ((pqa, 0, CA), (pqb, CA, CB)):
                    for j in range(7):
                        nc.tensor.matmul(p[:], qdw8[:, 2 * j:2 * j + 2, m0:m0 + mw],
                                         y8[:, 2 * j:2 * j + 2, :],
                                         start=(j == 0), stop=(j == 6),
                                         perf_mode=mybir.MatmulPerfMode.DoubleRow)
                qe_a = tpp.tile([CA, 512], dt.bfloat16, tag="qe_a")
                qe_b = tpp.tile([CB, 512], dt.bfloat16, tag="qe_b")
                nc.any.tensor_copy(qe_a[:], pqa[:])
                nc.any.tensor_copy(qe_b[:], pqb[:])
                nc.sync.dma_start_transpose(qt_full[:, 4 * i:4 * (i + 1), 0:CA], qe_a[:])
                _tp_last = nc.sync.dma_start_transpose(
                    qt_full[:, 4 * i:4 * (i + 1), CA:C], qe_b[:])
                sq = evp.tile([CA, 512], dt.bfloat16, tag="sqscr")
                nc.scalar.activation(sq[:, 0:512], qe_a[:], Act.Square,
                                     accum_out=ssq_a[:, i:i + 1])
                nc.scalar.activation(sq[0:CB, 0:512], qe_b[:], Act.Square,
                                     accum_out=ssq_b[:, i:i + 1])
                # k-upper/v-upper depthwise via PE diag (frees VectorE for k-lower)
                pdm = psp.tile([CA, 512], dt.float32, tag="mm")
                for t, (ky, kx) in enumerate(taps):
                    rhs_m = kv1b[:, 4 * i + ky:4 * i + ky + 4, kx:kx + W]
                    nc.tensor.matmul(pdm[:], dwdiag2[:, t, :], rhs_m,
                                     start=(t == 0), stop=(t == 8))
                nc.any.tensor_copy(kvb_out[:, s0:s0 + 512], pdm[:])
                squ = evp.tile([CA, 512], dt.bfloat16, tag="sqscr")
                nc.scalar.activation(squ[CB:CA], kvb_out[CB:CA, s0:s0 + 512],
                                     Act.Square, accum_out=ssk_u[CB:CA, i:i + 1])

            # ---------- phase 5: reduce the sums of squares (squares were
            # issued incrementally above) ----------
            ssq = smp.tile([CA, 2], dt.float32)
            ssk = smp.tile([CA, 2], dt.float32)
            nc.vector.tensor_reduce(ssq[:, 0:1], ssq_a[:], mybir.AxisListType.X, Alu.add)
            nc.vector.tensor_reduce(ssq[0:CB, 1:2], ssq_b[:], mybir.AxisListType.X, Alu.add)
            nc.vector.tensor_reduce(ssk[:, 0:1], ssk_a[:], mybir.AxisListType.X, Alu.add)
            nc.vector.tensor_reduce(ssk[CB:CA, 1:2], ssk_u[CB:CA], mybir.AxisListType.X, Alu.add)
            # warm the Sqrt activation table for the norm chain below
            nc.scalar.sqrt(warm[:], warm[:])

            # ---------- phase 6a: norms all-reduce -- issued as soon as the
            # sums of squares are reduced, so it (and the gpsimd collective
            # library cold-start) runs entirely in QK^T's shadow; fq/fk are
            # then precomputed before the logits arrive ----------
            cin1 = dramp.tile([2, C], dt.float32)
            cout1 = dramp.tile([2, C], dt.float32)
            nc.gpsimd.dma_start(cin1[0:1, 0:CA].rearrange("o c -> c o"), ssq[:, 0:1])
            nc.gpsimd.dma_start(cin1[0:1, CA:C].rearrange("o c -> c o"), ssq[0:CB, 1:2])
            nc.gpsimd.dma_start(cin1[1:2, 0:CA].rearrange("o c -> c o"), ssk[:, 0:1])
            nc.gpsimd.dma_start(cin1[1:2, CA:C].rearrange("o c -> c o"), ssk[CB:CA, 1:2])
            nc.gpsimd.collective_compute(
                "AllReduce", Alu.add,
                replica_groups=[[0, 1], [2, 3], [4, 5], [6, 7]],
                ins=[cin1[:].opt()], outs=[cout1[:].opt()])
            fq = smp.tile([HC, HEADS], dt.float32)
            fk = smp.tile([1, C], dt.float32)
            nc.gpsimd.dma_start(fq[:], cout1[0:1, :].rearrange("o (h c) -> c (o h)", h=HEADS))
            nc.gpsimd.dma_start(fk[:], cout1[1:2, :])
            nc.scalar.sqrt(fq[:], fq[:])
            nc.vector.reciprocal(fq[:], fq[:])
            nc.vector.tensor_tensor(fq[:], fq[:], temp_s[:], Alu.mult)
            fk32 = smp.tile([HC, C], dt.float32)
            nc.gpsimd.partition_broadcast(fk32[:], fk[:])
            nc.scalar.sqrt(fk32[:], fk32[:])
            nc.vector.reciprocal(fk32[:], fk32[:])
            # warm the Exp table last so the post-collective exp is warm
            nc.scalar.activation(warm[:], warm[:], Act.Exp)


            # ---------- phase 4b: QK^T with just-in-time k transposes; the
            # deferred v-lower conv chunks fill the PE's kt-wait gaps ----------
            from concourse.tile_rust import add_dep_helper as _adh
            for i in range(NCH):
                s0 = 512 * i
                kt4 = tpp.tile([128, 4, C], dt.bfloat16, tag="kt")
                # alternate HWDGE queues with an explicit serialization chain:
                # the xbar phases never overlap (corruption constraint) but the
                # per-instruction sequencer setup pipelines across queues
                _q = nc.sync if i % 2 == 0 else nc.scalar
                _t1 = _q.dma_start_transpose(kt4[:, :, 0:CA], ka[:, s0:s0 + 512])
                _adh(_tp_last.ins, _t1.ins, sync=True,
                     reason="serialize xbar transposes across queues")
                _tp_last = _q.dma_start_transpose(kt4[:, :, CA:C],
                                                  kvb_out[CB:CA, s0:s0 + 512])
                for j in range(4):
                    nc.tensor.matmul(attn_pa[:], qt_full[:, 4 * i + j, 0:CA], kt4[:, j, :],
                                     start=(i == 0 and j == 0),
                                     stop=(i == NCH - 1 and j == 3),
                                     skip_group_check=True)
                    qkt_last = nc.tensor.matmul(
                        attn_pb[:], qt_full[:, 4 * i + j, CA:C], kt4[:, j, :],
                        start=(i == 0 and j == 0),
                        stop=(i == NCH - 1 and j == 3),
                        skip_group_check=True)
                c0, c1 = PCH[i]
                n = c1 - c0
                p2 = psp.tile([CA, 512], dt.float32, tag="mm")
                nc.tensor.matmul(p2[:, :n], kvw8[:, :, 256:384], x8s[:, :, c0:c1],
                                 start=True, stop=True,
                                 perf_mode=mybir.MatmulPerfMode.DoubleRow)
                nc.any.tensor_copy(flat(v1a)[:, c0:c1], p2[:, :n])
            c0, c1 = PCH[16]
            n = c1 - c0
            p2 = psp.tile([CA, 512], dt.float32, tag="mm")
            nc.tensor.matmul(p2[:, :n], kvw8[:, :, 256:384], x8s[:, :, c0:c1],
                             start=True, stop=True,
                             perf_mode=mybir.MatmulPerfMode.DoubleRow)
            nc.any.tensor_copy(flat(v1a)[:, c0:c1], p2[:, :n])
            # v-lower rows 0:32 on DVE (rows 32:64 run on PE during the collective)
            dw(va, v1a, dw_va, 0, 16)
            dw(va, v1a, dw_va, 16, 16)

            # preload the residual x into SBUF, aliasing the dead x8s/qt_full
            # slots: no DMA loads then compete with the collective window or
            # sit on the phase-9 tail. b-half packed two row-blocks deep.
            xca_s = bigp.tile([CA, S_IN], dt.bfloat16, tag="x8s")
            xcb_s = bigp.tile([CA, S_IN // 2], dt.bfloat16, tag="qt_full")
            nc.scalar.dma_start(xca_s[:], x_ctr_t.ap()[0:CA])
            nc.scalar.dma_start(xcb_s[0:CB, :], x_ctr_t.ap()[CA:C, 0:4096])
            nc.scalar.dma_start(xcb_s[CB:CA, :], x_ctr_t.ap()[CA:C, 4096:S_IN])

            # ---------- phase 4c: depthwise on v-lower rows 32:64 via PE diag ----------
            # Explicitly ordered after the QK^T stop so the attention logits
            # (and thus the AllReduce) aren't delayed; va-diag then overlaps
            # the collective instead of displacing it. (Rows 0:32 ran on DVE.)
            from concourse.tile_rust import add_dep_helper
            first_va = None
            for i in range(8, NCH):
                r0 = 4 * i
                pd = psp.tile([CA, 512], dt.float32, tag="mm")
                for t, (ky, kx) in enumerate(taps):
                    rhs = v1a[:, r0 + ky:r0 + ky + 4, kx:kx + W]
                    mi = nc.tensor.matmul(pd[:], dwdiag[:, t, :], rhs,
                                          start=(t == 0), stop=(t == 8))
                    if first_va is None:
                        first_va = mi
                nc.any.tensor_copy(va[:, 512 * i:512 * (i + 1)], pd[:])
            add_dep_helper(qkt_last.ins, first_va.ins, sync=False,
                           reason="finish QK^T before v-diag so the AllReduce starts early")
            _va_last = mi

            # ---------- phase 6b: logits all-reduce (warm library) ----------
            attn_sa = smp.tile([CA, C], dt.float32)
            attn_sb = smp.tile([CB, C], dt.float32)
            nc.any.tensor_copy(attn_sa[:], attn_pa[:])
            nc.any.tensor_copy(attn_sb[:], attn_pb[:])
            cin = dramp.tile([HC, C], dt.float32)
            cout = dramp.tile([HC, C], dt.float32)
            pengs = (nc.scalar, nc.sync)
            for h in range(HEADS):
                src = attn_sa if h < 4 else attn_sb
                r = HC * (h % 4)
                pengs[h % 2].dma_start(cin[0:HC, HC * h:HC * (h + 1)],
                                       src[r:r + HC, HC * h:HC * (h + 1)])
            nc.gpsimd.collective_compute(
                "AllReduce", Alu.add,
                replica_groups=[[0, 1], [2, 3], [4, 5], [6, 7]],
                ins=[cin[:].opt()], outs=[cout[:].opt()])

            # PE heater: dead matmuls into the already-extracted attn psum keep
            # the HAM clock at 8/8 through the AllReduce window so the tail
            # matmuls start warm. Results are never read (WAR on attn_pa gates
            # the start until after the extraction copies).
            for _ in range(40):
                nc.tensor.matmul(attn_pa, dwdiag[:, 0, :], va[:, 0:C],
                                 start=True, stop=True, skip_group_check=True)

            # ---------- phase 7: softmax (fq/fk already computed) ----------
            attn_f = smp.tile([HC, HEADS, HC], dt.float32)
            nc.scalar.dma_start(attn_f[:], cout[0:HC, :].rearrange("p (h c) -> p h c", h=HEADS))
            nc.vector.tensor_tensor(attn_f[:], attn_f[:],
                                    fq[:, :, None].to_broadcast((HC, HEADS, HC)), Alu.mult)
            nc.vector.tensor_tensor(attn_f[:], attn_f[:],
                                    fk32[:].rearrange("p (h c) -> p h c", h=HEADS), Alu.mult)
            ex = smp.tile([HC, HEADS, HC], dt.float32)
            nc.scalar.activation(ex[:], attn_f[:], Act.Exp)
            sm = smp.tile([HC, HEADS], dt.float32)
            nc.vector.tensor_reduce(sm[:], ex[:], mybir.AxisListType.X, Alu.add)
            nc.vector.reciprocal(sm[:], sm[:])
            nc.vector.tensor_tensor(ex[:], ex[:],
                                    sm[:, :, None].to_broadcast((HC, HEADS, HC)), Alu.mult)
            attn_bf = smp.tile([HC, HEADS, HC], dt.bfloat16)
            nc.vector.tensor_copy(attn_bf[:], ex[:])
            # block-diagonal NON-transposed attn: bd[i, j] = attn[i, j]
            bd_a = smp.tile([CA, CA], dt.bfloat16)
            bd_b = smp.tile([CB, CB], dt.bfloat16)
            nc.vector.memset(bd_a[:], 0.0)
            nc.vector.memset(bd_b[:], 0.0)
            qengs = (nc.sync, nc.gpsimd, nc.scalar)
            for h in range(HEADS):
                if h < 4:
                    qengs[h % 3].dma_start(bd_a[HC * h:HC * (h + 1), HC * h:HC * (h + 1)],
                                           attn_bf[:, h, :])
                else:
                    j = h - 4
                    qengs[h % 3].dma_start(bd_b[HC * j:HC * (j + 1), HC * j:HC * (j + 1)],
                                           attn_bf[:, h, :])

            # ---------- phase 8: fuse Wc^T = (proj @ attn_bd)^T = A^T-free form
            # WcT[j, o] = sum_i bd[i, j] * projT[i, o]  (j = v channel)
            pw1 = psp.tile([CA, C], dt.float32, tag="mm")
            pw2 = psp.tile([CB, C], dt.float32, tag="mm")
            nc.tensor.matmul(pw1[:], bd_a[:], projw_a[:, 0:C], start=True, stop=True)
            nc.tensor.matmul(pw2[:], bd_b[:], projw_b[0:CB, 0:C], start=True, stop=True)
            wcT_a = smp.tile([CA, C], dt.bfloat16)
            wcT_b = smp.tile([CA, C], dt.bfloat16)  # rows 64:128 zero (cancel k-upper)
            nc.vector.memset(wcT_b[:], 0.0)
            nc.scalar.copy(wcT_a[:], pw1[:])
            nc.scalar.copy(wcT_b[0:CB, :], pw2[:])

            # ---------- phase 9: out = Wc @ v, + residual ----------
            for i in range(NCH):
                s0 = 512 * i
                ppa = psp.tile([CA, 512], dt.float32, tag="mm")
                ppb = psp.tile([CB, 512], dt.float32, tag="mm")
                nc.tensor.matmul(ppa[:], wcT_a[:, 0:CA], va[:, s0:s0 + 512], start=True, stop=False)
                # rhs rows 64:128 hold dw'd k-upper; wcT_b zero rows cancel them
                nc.tensor.matmul(ppa[:], wcT_b[:, 0:CA], kvb_out[:, s0:s0 + 512], start=False, stop=True)
                nc.tensor.matmul(ppb[:], wcT_a[:, CA:C], va[:, s0:s0 + 512], start=True, stop=False)
                nc.tensor.matmul(ppb[:], wcT_b[:, CA:C], kvb_out[:, s0:s0 + 512], start=False, stop=True)
                xca_v = xca_s[:, s0:s0 + 512]
                if i < 8:
                    xcb_v = xcb_s[0:CB, s0:s0 + 512]
                else:
                    xcb_v = xcb_s[CB:CA, s0 - 4096:s0 - 4096 + 512]
                nc.vector.scalar_tensor_tensor(xca_v, ppa[:], -1.0, xca_v, Alu.mult, Alu.add)
                nc.vector.scalar_tensor_tensor(xcb_v, ppb[:], -1.0, xcb_v, Alu.mult, Alu.add)
                nc.sync.dma_start(out_t.ap()[0:CA, s0:s0 + 512], xca_v)
                nc.gpsimd.dma_start(out_t.ap()[CA:C, s0:s0 + 512], xcb_v)

    nc.compile()
    return nc


def _host_prep(inputs):
    x = np.asarray(inputs["x"], dtype=np.float32)
    y = np.asarray(inputs["y"], dtype=np.float32)
    kv_w = np.asarray(inputs["kv_w"], dtype=np.float32)[:, :, 0, 0]
    kv_dw = np.asarray(inputs["kv_dw_w"], dtype=np.float32)[:, 0]
    q_w = np.asarray(inputs["q_w"], dtype=np.float32)[:, :, 0, 0]
    q_dw = np.asarray(inputs["q_dw_w"], dtype=np.float32)
    proj_w = np.asarray(inputs["proj_w"], dtype=np.float32)[:, :, 0, 0]
    temp = np.asarray(inputs["temperature"], dtype=np.float32)[:, 0, 0]

    def kpad(a):  # [192, M] -> [256, M] with zero rows
        return np.concatenate([a, np.zeros((CP - C, a.shape[1]), a.dtype)], 0)

    f8 = ml_dtypes.float8_e4m3
    S_KV = 64.0

    def to8(a):
        return np.ascontiguousarray(np.clip(a, -224, 224).astype(f8))

    # kv output-channel permutation: [k 0:128 | v 128:192 ; k 128:192 | v 0:128]
    perm = np.concatenate([np.arange(0, 128), np.arange(320, 384),
                           np.arange(128, 192), np.arange(192, 320)])
    # fp8 DoubleRow weights [K=128, plane, out]: plane0 = in-ch 0:128,
    # plane1 = in-ch 128:192 (+zero rows). Scaled by S_KV; k rows scale-cancel
    # in the l2 norm, v rows are unscaled by folding 1/S_KV into proj_w.
    kvwp = kv_w[perm].T * S_KV          # [192, 384]
    kvw8 = np.zeros((128, 2, 2 * C), np.float32)
    kvw8[:, 0, :] = kvwp[0:128]
    kvw8[0:64, 1, :] = kvwp[128:192]
    kvw8 = to8(kvw8)
    # fold q_w (1x1) into the full 3x3: W2[o,i,ky,kx] = sum_c q_dw[o,c,..] q_w[c,i]
    W2 = np.einsum('ocyx,ci->oiyx', q_dw, q_w)
    # per-output-channel scaling (cancels exactly in the q l2 norm)
    sq = 120.0 / np.abs(W2).max(axis=(1, 2, 3))
    W2s = W2 * sq[:, None, None, None]
    # 14 K-planes matching y_f8: 0-8 = in-ch 0:128 tap t; 9+s = in-ch 128:192
    # [tap 2s ; tap 2s+1]
    qdw8 = np.zeros((128, 14, C), np.float32)
    for t in range(9):
        ky, kx = t // 3, t % 3
        qdw8[:, t, :] = W2s[:, 0:128, ky, kx].T
    for s in range(5):
        t0, t1 = 2 * s, 2 * s + 1
        qdw8[0:64, 9 + s, :] = W2s[:, 128:192, t0 // 3, t0 % 3].T
        if t1 < 9:
            qdw8[64:128, 9 + s, :] = W2s[:, 128:192, t1 // 3, t1 % 3].T
    qdw8 = to8(qdw8)
    kdw = kv_dw[:C].reshape(C, 9)
    vdw = kv_dw[C:].reshape(C, 9)
    dw_all = np.ascontiguousarray(np.concatenate(
        [kdw[0:128], vdw[128:192], kdw[128:192], vdw[0:128]], 0))
    dw_diag = np.zeros((9, 128, 128), np.float32)
    dw_diag2 = np.zeros((9, 128, 128), np.float32)
    mixdw = np.concatenate([vdw[128:192], kdw[128:192]], 0)
    for t in range(9):
        np.fill_diagonal(dw_diag[t], vdw[0:128, t])
        np.fill_diagonal(dw_diag2[t], mixdw[:, t])
    dw_diag = np.ascontiguousarray(dw_diag).astype(bf16)
    dw_diag2 = np.ascontiguousarray(dw_diag2).astype(bf16)
    proj_wT = np.ascontiguousarray(kpad(proj_w.T / S_KV)).astype(bf16)
    temp2 = np.ascontiguousarray(np.broadcast_to(temp.reshape(1, HEADS), (HC, HEADS)))

    def padded(arr, b, s):
        r0 = HP * s
        p = np.zeros((C, PH, PW), np.float32)
        lo, hi = max(r0 - 1, 0), min(r0 + HP + 1, H)
        p[:, lo - r0 + 1:hi - r0 + 1, 1:W + 1] = arr[b, :, lo:hi, :]
        return p

    def shard_x8(arr, b, s):
        p = padded(arr, b, s)
        x8 = np.zeros((128, 2, 8592), np.float32)
        x8[:, 0, :S_PAD] = p[0:128].reshape(128, S_PAD)
        x8[0:64, 1, :S_PAD] = p[128:192].reshape(64, S_PAD)
        return to8(x8)

    def shard_y8(arr, b, s):
        p = padded(arr, b, s)
        y8 = np.zeros((128, 14, HP, W), np.float32)
        for t in range(9):
            ky, kx = t // 3, t % 3
            y8[:, t] = p[0:128, ky:ky + HP, kx:kx + W]
        for sp in range(5):
            t0, t1 = 2 * sp, 2 * sp + 1
            ky, kx = t0 // 3, t0 % 3
            y8[0:64, 9 + sp] = p[128:192, ky:ky + HP, kx:kx + W]
            if t1 < 9:
                ky, kx = t1 // 3, t1 % 3
                y8[64:128, 9 + sp] = p[128:192, ky:ky + HP, kx:kx + W]
        # chunk-major: [128, 16 chunks, 14 planes, 512]
        return to8(np.ascontiguousarray(
            y8.reshape(128, 14, S_IN // 512, 512).transpose(0, 2, 1, 3)))

    in_maps = []
    for core in range(NCORES):
        b, s = core // 2, core % 2
        r0 = HP * s
        in_maps.append({
            "x_f8": shard_x8(x, b, s),
            "y_f8": shard_y8(y, b, s),
            "x_ctr": np.ascontiguousarray(
                x[b, :, r0:r0 + HP, :].reshape(C, S_IN).astype(bf16)),
            "kv_wT": kvw8, "qdw_T": qdw8,
            "dw_all": dw_all, "dw_diag": dw_diag, "dw_diag2": dw_diag2,
            "proj_wT": proj_wT,
            "temp": temp2,
        })
    return in_maps


LAST_RESULT = None


def kernel(**inputs):
    global LAST_RESULT
    from concourse.bass_utils import run_bass_kernel_spmd

    if "nc" not in _cache:
        _cache["nc"] = _build()
    nc = _cache["nc"]
    in_maps = _host_prep(inputs)
    res = run_bass_kernel_spmd(nc, in_maps, core_ids=list(range(NCORES)))
    LAST_RESULT = res
    out = np.empty((B, C, H, W), np.float32)
    for core in range(NCORES):
        b, s = core // 2, core % 2
        out[b, :, HP * s:HP * (s + 1), :] = \
            res.results[core]["out"].reshape(C, HP, W).astype(np.float32)
    return out



# revision 85
# speedup vs baseline: 1.0570x; 1.0570x over previous
"""Trainium2 Bass kernel for nn_Attention_57672820850902.

Channel-attention block (XCA-style):
  kv = dwconv3x3(conv1x1(x)); k, v = split(kv)
  q  = conv3x3_full(conv1x1(y))
  q, k l2-normalized per channel row; attn = softmax(q @ k^T * temp) per head
  out = x - conv1x1(attn @ v)

Sharding: 8 cores = 4 batches x 2 spatial halves (64 rows + 1-row halo).
Cross-core traffic is two tiny pairwise AllReduces (norm sums, then the
per-head 32x32 attention logits).

Architecture (evolved from the v1 baseline; ~371us -> ~280-315us):
- The q 1x1 conv is FOLDED into the 3x3 on the host (W2 = q_dw @ q_w); the
  fused 3x3 runs as fp8-e4m3 DoubleRow matmuls (K=256/instr) over 14
  host-prebuilt shifted K-planes on the compact grid (7 tap-pair matmuls
  per M-half per 512-chunk). Per-output-channel weight scales cancel
  exactly in the l2 norm. The kv 1x1 conv is likewise fp8 DoubleRow.
- proj @ blockdiag(attn) is fused into one 192x192 matrix WcT on-device,
  so attn@v and the projection are a single matmul pass.
- Depthwise 3x3: k-lower on VectorE (row-quartered STT taps), k-upper/
  v-upper and v-lower via PE diagonal matmuls (v-lower rows 0:32 on DVE);
  the PE-diag halves run inside the phase-4/collective shadows.
- The AllReduce is SPLIT: norms first (issued before QK^T; hides the
  ~11us gpsimd collective-library cold start and lets fq/fk precompute
  off-path), then the logits on a warm library (~7us). Exp/Sqrt act
  tables are pre-warmed; the k-norm reciprocal runs after the partition
  broadcast (32 lanes, not 1).
- DMA: x/y ship as fp8 (chunk-major y planes: one contiguous descriptor
  per partition); the residual x preloads into SBUF aliasing the dead
  x8s/qt_full slots; residual input and output are bf16 (rel err ~5.7e-3
  vs the 2e-2 gate, dominated by those bf16 roundings).
- q/k spatial transposes for QK^T remain 512-wide xbar DMA transposes,
  serialized on the sync queue (concurrent xbar transposes from both
  HWDGE queues corrupt tiles). Measured dead ends, do not retry: PE
  transposes inside the QK^T stream (3x tried, ~60us WORSE: they break
  the attention accumulation-group flow), fp8 xbar transposes (API
  requires 2-byte dtypes), dummy collective warm-up, kt-interleave
  (wash), manual evac engine pinning (nc.any wins).
- Known remaining levers: orientation-swapped phase 4 under DoubleRow
  (would emit q pre-transposed, deleting the 40us qt chain; needs a
  partition-dim ssq reduction), and raw pairwise remote_dma in place of
  the second collective (~10us).
"""

import os
import numpy as np
import ml_dtypes

B, C, H, W, HEADS = 4, 192, 128, 128, 6
HC = C // HEADS                      # 32 channels per head
HP = H // 2                          # 64 rows per core
PH, PW = HP + 2, W + 2               # 66 x 130 padded shard
S_PAD = PH * PW                      # 8580
S_IN = HP * W                        # 8192
NCORES = 8
CA, CB = 128, 64                     # channel tile split of 192
CP = 256                             # K-padded channel count

bf16 = ml_dtypes.bfloat16

_cache = {}


def _pad_chunks():
    bounds = list(range(0, S_PAD, 512)) + [S_PAD]
    return list(zip(bounds[:-1], bounds[1:]))


def _build():
    import concourse.bass as bass
    import concourse.mybir as mybir
    import concourse.tile as tile
    from concourse import bacc

    dt = mybir.dt
    Alu = mybir.AluOpType
    Act = mybir.ActivationFunctionType

    nc = bacc.Bacc("TRN2", target_bir_lowering=False, debug=False,
                   num_devices=NCORES)

    # ---- per-core inputs ----
    # x for the kv conv as fp8 DoubleRow planes: plane0 = ch 0:128,
    # plane1 = ch 128:192 (+zero rows), both on the padded 66x130 grid
    # S_PAD rounded to 8592 (mult of 16) so the DoubleRow plane stride is legal
    x8_t = nc.dram_tensor("x_f8", [CA, 2, 8592], dt.float8e4, kind="ExternalInput")
    # y for the folded q 3x3 conv as 14 fp8 K-planes on the compact 64x128
    # output grid (shifts pre-baked on host): planes 0-8 = ch 0:128 taps 0-8;
    # planes 9+s = ch 128:192 [tap 2s rows 0:64 ; tap 2s+1 rows 64:128].
    # Chunk-major layout so each 512-col load is one contiguous descriptor
    # per partition.
    y8_t = nc.dram_tensor("y_f8", [CA, S_IN // 512, 14, 512], dt.float8e4,
                          kind="ExternalInput")
    x_ctr_t = nc.dram_tensor("x_ctr", [C, S_IN], dt.bfloat16, kind="ExternalInput")
    # ---- weights (same on all cores; K rows host-padded to 256) ----
    # kv_wT columns host-permuted to [k 0:128 | v 128:192 ; k 128:192 | v 0:128]
    kvw_t = nc.dram_tensor("kv_wT", [CA, 2, 2 * C], dt.float8e4, kind="ExternalInput")
    qdw_t = nc.dram_tensor("qdw_T", [CA, 14, C], dt.float8e4, kind="ExternalInput")
    # dw_all rows: [0:128]=k 0:128 | [128:192]=v 128:192 ; [192:256]=k 128:192 | [256:384]=v 0:128
    dw_t = nc.dram_tensor("dw_all", [384, 9], dt.float32, kind="ExternalInput")
    dwdiag_t = nc.dram_tensor("dw_diag", [9, 128, 128], dt.bfloat16, kind="ExternalInput")
    dwdiag2_t = nc.dram_tensor("dw_diag2", [9, 128, 128], dt.bfloat16, kind="ExternalInput")
    projw_t = nc.dram_tensor("proj_wT", [CP, C], dt.bfloat16, kind="ExternalInput")
    temp_t = nc.dram_tensor("temp", [HC, HEADS], dt.float32, kind="ExternalInput")
    out_t = nc.dram_tensor("out", [C, S_IN], dt.bfloat16, kind="ExternalOutput")

    PCH = _pad_chunks()
    taps = [(ky, kx) for ky in range(3) for kx in range(3)]
    NCH = S_IN // 512                # 16 inner chunks

    with tile.TileContext(nc) as tc:
        with tc.tile_pool(name="w", bufs=1) as wp, \
             tc.tile_pool(name="big", bufs=1) as bigp, \
             tc.tile_pool(name="io", bufs=3) as iop, \
             tc.tile_pool(name="io2", bufs=4) as iop2, \
             tc.tile_pool(name="tp", bufs=7) as tpp, \
             tc.tile_pool(name="ev", bufs=2) as evp, \
             tc.tile_pool(name="small", bufs=1) as smp, \
             tc.tile_pool(name="ps", bufs=6, space="PSUM") as psp, \
             tc.tile_pool(name="psattn", bufs=1, space="PSUM") as psattn, \
             tc.tile_pool(name="dram", bufs=1, space="DRAM") as dramp:

            # ---------- weights to SBUF ----------
            kvw8 = wp.tile([CA, 2, 2 * C], dt.float8e4)
            nc.sync.dma_start(kvw8[:], kvw_t.ap())
            qdw8 = wp.tile([CA, 14, C], dt.float8e4)
            nc.scalar.dma_start(qdw8[:], qdw_t.ap())
            dwk_a = wp.tile([CA, 9], dt.float32)
            dw_va = wp.tile([CA, 9], dt.float32)
            nc.sync.dma_start(dwk_a[:], dw_t.ap()[0:128])
            nc.sync.dma_start(dw_va[:], dw_t.ap()[256:384])
            dwdiag = wp.tile([128, 9, 128], dt.bfloat16)
            nc.scalar.dma_start(dwdiag[:], dwdiag_t.ap().rearrange("t k m -> k t m"))
            dwdiag2 = wp.tile([128, 9, 128], dt.bfloat16)
            nc.scalar.dma_start(dwdiag2[:], dwdiag2_t.ap().rearrange("t k m -> k t m"))
            projw_a = wp.tile([CA, C], dt.bfloat16)
            projw_b = wp.tile([CA, C], dt.bfloat16)
            nc.scalar.dma_start(projw_a[:], projw_t.ap()[0:CA])
            nc.scalar.dma_start(projw_b[:], projw_t.ap()[CA:CP])
            temp_s = wp.tile([HC, HEADS], dt.float32)
            nc.sync.dma_start(temp_s[:], temp_t.ap())
            warm = wp.tile([1, 1], dt.float32)
            nc.vector.memset(warm[:], 1.0)

            # ---------- persistent intermediates ----------
            k1a = bigp.tile([CA, PH, PW], dt.bfloat16, tag="k1a")
            # kv1b: rows 0:64 = v ch 128:192 ("v1b"), rows 64:128 = k ch 128:192 ("k1u")
            kv1b = bigp.tile([CA, PH, PW], dt.bfloat16, tag="kv1b")
            v1a = bigp.tile([CA, PH, PW], dt.bfloat16, tag="v1a")
            qt_full = bigp.tile([128, 64, C], dt.bfloat16, tag="qt_full")
            ka = bigp.tile([CA, S_IN], dt.bfloat16, tag="ka")
            # kvb_out: rows 0:64 = v ch 128:192 dw'd, rows 64:128 = k ch 128:192 dw'd
            kvb_out = bigp.tile([CA, S_IN], dt.bfloat16, tag="kvb_out")
            # va reuses the k1a slot (k1a dead after the k depthwise)
            va = bigp.tile([CA, S_IN], dt.bfloat16, tag="k1a")


            attn_pa = psattn.tile([CA, C], dt.float32, tag="attnA")
            attn_pb = psattn.tile([CB, C], dt.float32, tag="attnB")

            def flat(t):
                return t[:].rearrange("p h w -> p (h w)")

            # ---------- phase 1: kv1 = kv_w @ x (padded grid, fp8 DoubleRow) ----------
            #   psum0 = k[0:128]; psum1 = [v 128:192 ; k 128:192]; psum2 = v[0:128]
            # x8 loaded whole (17KB/partition), split across 4 queues so the
            # PE is never DMA-gated here.
            x8s = bigp.tile([CA, 2, 8592], dt.float8e4, tag="x8s")
            for qeng, lo, hi in ((nc.gpsimd, 0, 4296), (nc.sync, 4296, 8592)):
                qeng.dma_start(x8s[:, 0, lo:hi], x8_t.ap()[:, 0, lo:hi])
                qeng.dma_start(x8s[:, 1, lo:hi], x8_t.ap()[:, 1, lo:hi])
            for c0, c1 in PCH:
                n = c1 - c0
                p0 = psp.tile([CA, 512], dt.float32, tag="mm")
                p1 = psp.tile([CA, 512], dt.float32, tag="mm")
                for p, m0 in ((p0, 0), (p1, 128)):
                    nc.tensor.matmul(p[:, :n], kvw8[:, :, m0:m0 + 128],
                                     x8s[:, :, c0:c1],
                                     start=True, stop=True,
                                     perf_mode=mybir.MatmulPerfMode.DoubleRow)
                nc.any.tensor_copy(flat(k1a)[:, c0:c1], p0[:, :n])
                nc.any.tensor_copy(flat(kv1b)[:, c0:c1], p1[:, :n])

            # ---------- depthwise 3x3 taps (FMA on VectorE), row-chunked so
            # downstream per-chunk consumers (kt transposes, proj) unblock early
            def dw(dst, src, wsc, h0, nh):
                first = True
                for t, (ky, kx) in enumerate(taps):
                    shifted = src[:, h0 + ky:h0 + ky + nh, kx:kx + W]
                    d = dst[:].rearrange("p (h w) -> p h w", w=W)[:, h0:h0 + nh]
                    if first:
                        nc.vector.tensor_scalar(d, shifted, wsc[:, t:t + 1], None, Alu.mult)
                        first = False
                    else:
                        nc.vector.scalar_tensor_tensor(
                            d, shifted, wsc[:, t:t + 1], d, Alu.mult, Alu.add)

            # ---------- phase 2: depthwise on k-lower (VectorE, quartered);
            # k-upper/v-upper moved to PE diag inside the phase-4 loop ----------
            ssk_a = smp.tile([CA, 8], dt.float32)
            ssk_u = smp.tile([CA, NCH], dt.float32)   # rows 64:128 active
            for qtr in range(4):
                dw(ka, k1a, dwk_a, 16 * qtr, 16)
                for h in range(2):
                    i = 2 * qtr + h
                    sq = evp.tile([CA, 1024], dt.bfloat16, tag="sqscr")
                    nc.scalar.activation(sq[:], ka[:, 1024 * i:1024 * (i + 1)],
                                         Act.Square, accum_out=ssk_a[:, i:i + 1])


            # ---------- phase 4: q = folded 3x3 conv (fp8 DoubleRow, 7 K-pair
            # matmuls per M-half); evac -> transpose to qt_full ----------
            ssq_a = smp.tile([CA, 16], dt.float32)
            ssq_b = smp.tile([CB, 16], dt.float32)
            for i in range(NCH):
                s0 = 512 * i
                y8 = iop2.tile([CA, 14, 512], dt.float8e4, tag="y8")
                nc.scalar.dma_start(y8[:, 0:7], y8_t.ap()[:, i, 0:7])
                nc.gpsimd.dma_start(y8[:, 7:14], y8_t.ap()[:, i, 7:14])
                pqa = psp.tile([CA, 512], dt.float32, tag="mm")
                pqb = psp.tile([CB, 512], dt.float32, tag="mm")
                for p, m0, mw in ((pqa, 0, CA), (pqb, CA, CB)):
                    for j in range(7):
                        nc.tensor.matmul(p[:], qdw8[:, 2 * j:2 * j + 2, m0:m0 + mw],
                                         y8[:, 2 * j:2 * j + 2, :],
                                         start=(j == 0), stop=(j == 6),
                                         perf_mode=mybir.MatmulPerfMode.DoubleRow)
                qe_a = tpp.tile([CA, 512], dt.bfloat16, tag="qe_a")
                qe_b = tpp.tile([CB, 512], dt.bfloat16, tag="qe_b")
                nc.any.tensor_copy(qe_a[:], pqa[:])
                nc.any.tensor_copy(qe_b[:], pqb[:])
                nc.sync.dma_start_transpose(qt_full[:, 4 * i:4 * (i + 1), 0:CA], qe_a[:])
                nc.sync.dma_start_transpose(qt_full[:, 4 * i:4 * (i + 1), CA:C], qe_b[:])
                sq = evp.tile([CA, 512], dt.bfloat16, tag="sqscr")
                nc.scalar.activation(sq[:, 0:512], qe_a[:], Act.Square,
                                     accum_out=ssq_a[:, i:i + 1])
                nc.scalar.activation(sq[0:CB, 0:512], qe_b[:], Act.Square,
                                     accum_out=ssq_b[:, i:i + 1])
                # k-upper/v-upper depthwise via PE diag (frees VectorE for k-lower)
                pdm = psp.tile([CA, 512], dt.float32, tag="mm")
                for t, (ky, kx) in enumerate(taps):
                    rhs_m = kv1b[:, 4 * i + ky:4 * i + ky + 4, kx:kx + W]
                    nc.tensor.matmul(pdm[:], dwdiag2[:, t, :], rhs_m,
                                     start=(t == 0), stop=(t == 8))
                nc.any.tensor_copy(kvb_out[:, s0:s0 + 512], pdm[:])
                squ = evp.tile([CA, 512], dt.bfloat16, tag="sqscr")
                nc.scalar.activation(squ[CB:CA], kvb_out[CB:CA, s0:s0 + 512],
                                     Act.Square, accum_out=ssk_u[CB:CA, i:i + 1])

            # ---------- phase 5: reduce the sums of squares (squares were
            # issued incrementally above) ----------
            ssq = smp.tile([CA, 2], dt.float32)
            ssk = smp.tile([CA, 2], dt.float32)
            nc.vector.tensor_reduce(ssq[:, 0:1], ssq_a[:], mybir.AxisListType.X, Alu.add)
            nc.vector.tensor_reduce(ssq[0:CB, 1:2], ssq_b[:], mybir.AxisListType.X, Alu.add)
            nc.vector.tensor_reduce(ssk[:, 0:1], ssk_a[:], mybir.AxisListType.X, Alu.add)
            nc.vector.tensor_reduce(ssk[CB:CA, 1:2], ssk_u[CB:CA], mybir.AxisListType.X, Alu.add)
            # warm the Sqrt activation table for the norm chain below
            nc.scalar.sqrt(warm[:], warm[:])

            # ---------- phase 6a: norms all-reduce -- issued as soon as the
            # sums of squares are reduced, so it (and the gpsimd collective
            # library cold-start) runs entirely in QK^T's shadow; fq/fk are
            # then precomputed before the logits arrive ----------
            cin1 = dramp.tile([2, C], dt.float32)
            cout1 = dramp.tile([2, C], dt.float32)
            nc.gpsimd.dma_start(cin1[0:1, 0:CA].rearrange("o c -> c o"), ssq[:, 0:1])
            nc.gpsimd.dma_start(cin1[0:1, CA:C].rearrange("o c -> c o"), ssq[0:CB, 1:2])
            nc.gpsimd.dma_start(cin1[1:2, 0:CA].rearrange("o c -> c o"), ssk[:, 0:1])
            nc.gpsimd.dma_start(cin1[1:2, CA:C].rearrange("o c -> c o"), ssk[CB:CA, 1:2])
            nc.gpsimd.collective_compute(
                "AllReduce", Alu.add,
                replica_groups=[[0, 1], [2, 3], [4, 5], [6, 7]],
                ins=[cin1[:].opt()], outs=[cout1[:].opt()])
            fq = smp.tile([HC, HEADS], dt.float32)
            fk = smp.tile([1, C], dt.float32)
            nc.gpsimd.dma_start(fq[:], cout1[0:1, :].rearrange("o (h c) -> c (o h)", h=HEADS))
            nc.gpsimd.dma_start(fk[:], cout1[1:2, :])
            nc.scalar.sqrt(fq[:], fq[:])
            nc.vector.reciprocal(fq[:], fq[:])
            nc.vector.tensor_tensor(fq[:], fq[:], temp_s[:], Alu.mult)
            fk32 = smp.tile([HC, C], dt.float32)
            nc.gpsimd.partition_broadcast(fk32[:], fk[:])
            nc.scalar.sqrt(fk32[:], fk32[:])
            nc.vector.reciprocal(fk32[:], fk32[:])
            # warm the Exp table last so the post-collective exp is warm
            nc.scalar.activation(warm[:], warm[:], Act.Exp)


            # ---------- phase 4b: QK^T with just-in-time k transposes; the
            # deferred v-lower conv chunks fill the PE's kt-wait gaps ----------
            for i in range(NCH):
                s0 = 512 * i
                kt4 = tpp.tile([128, 4, C], dt.bfloat16, tag="kt")
                nc.sync.dma_start_transpose(kt4[:, :, 0:CA], ka[:, s0:s0 + 512])
                nc.sync.dma_start_transpose(kt4[:, :, CA:C], kvb_out[CB:CA, s0:s0 + 512])
                for j in range(4):
                    nc.tensor.matmul(attn_pa[:], qt_full[:, 4 * i + j, 0:CA], kt4[:, j, :],
                                     start=(i == 0 and j == 0),
                                     stop=(i == NCH - 1 and j == 3),
                                     skip_group_check=True)
                    qkt_last = nc.tensor.matmul(
                        attn_pb[:], qt_full[:, 4 * i + j, CA:C], kt4[:, j, :],
                        start=(i == 0 and j == 0),
                        stop=(i == NCH - 1 and j == 3),
                        skip_group_check=True)
                c0, c1 = PCH[i]
                n = c1 - c0
                p2 = psp.tile([CA, 512], dt.float32, tag="mm")
                nc.tensor.matmul(p2[:, :n], kvw8[:, :, 256:384], x8s[:, :, c0:c1],
                                 start=True, stop=True,
                                 perf_mode=mybir.MatmulPerfMode.DoubleRow)
                nc.any.tensor_copy(flat(v1a)[:, c0:c1], p2[:, :n])
            c0, c1 = PCH[16]
            n = c1 - c0
            p2 = psp.tile([CA, 512], dt.float32, tag="mm")
            nc.tensor.matmul(p2[:, :n], kvw8[:, :, 256:384], x8s[:, :, c0:c1],
                             start=True, stop=True,
                             perf_mode=mybir.MatmulPerfMode.DoubleRow)
            nc.any.tensor_copy(flat(v1a)[:, c0:c1], p2[:, :n])
            # v-lower rows 0:32 on DVE (rows 32:64 run on PE during the collective)
            dw(va, v1a, dw_va, 0, 16)
            dw(va, v1a, dw_va, 16, 16)

            # preload the residual x into SBUF, aliasing the dead x8s/qt_full
            # slots: no DMA loads then compete with the collective window or
            # sit on the phase-9 tail. b-half packed two row-blocks deep.
            xca_s = bigp.tile([CA, S_IN], dt.bfloat16, tag="x8s")
            xcb_s = bigp.tile([CA, S_IN // 2], dt.bfloat16, tag="qt_full")
            nc.scalar.dma_start(xca_s[:], x_ctr_t.ap()[0:CA])
            nc.scalar.dma_start(xcb_s[0:CB, :], x_ctr_t.ap()[CA:C, 0:4096])
            nc.scalar.dma_start(xcb_s[CB:CA, :], x_ctr_t.ap()[CA:C, 4096:S_IN])

            # ---------- phase 4c: depthwise on v-lower rows 32:64 via PE diag ----------
            # Explicitly ordered after the QK^T stop so the attention logits
            # (and thus the AllReduce) aren't delayed; va-diag then overlaps
            # the collective instead of displacing it. (Rows 0:32 ran on DVE.)
            from concourse.tile_rust import add_dep_helper
            first_va = None
            for i in range(8, NCH):
                r0 = 4 * i
                pd = psp.tile([CA, 512], dt.float32, tag="mm")
                for t, (ky, kx) in enumerate(taps):
                    rhs = v1a[:, r0 + ky:r0 + ky + 4, kx:kx + W]
                    mi = nc.tensor.matmul(pd[:], dwdiag[:, t, :], rhs,
                                          start=(t == 0), stop=(t == 8))
                    if first_va is None:
                        first_va = mi
                nc.any.tensor_copy(va[:, 512 * i:512 * (i + 1)], pd[:])
            add_dep_helper(qkt_last.ins, first_va.ins, sync=False,
                           reason="finish QK^T before v-diag so the AllReduce starts early")
            _va_last = mi

            # ---------- phase 6b: logits all-reduce (warm library) ----------
            attn_sa = smp.tile([CA, C], dt.float32)
            attn_sb = smp.tile([CB, C], dt.float32)
            nc.any.tensor_copy(attn_sa[:], attn_pa[:])
            nc.any.tensor_copy(attn_sb[:], attn_pb[:])
            cin = dramp.tile([HC, C], dt.float32)
            cout = dramp.tile([HC, C], dt.float32)
            pengs = (nc.scalar, nc.sync)
            for h in range(HEADS):
                src = attn_sa if h < 4 else attn_sb
                r = HC * (h % 4)
                pengs[h % 2].dma_start(cin[0:HC, HC * h:HC * (h + 1)],
                                       src[r:r + HC, HC * h:HC * (h + 1)])
            nc.gpsimd.collective_compute(
                "AllReduce", Alu.add,
                replica_groups=[[0, 1], [2, 3], [4, 5], [6, 7]],
                ins=[cin[:].opt()], outs=[cout[:].opt()])

            # PE heater: dead matmuls into the already-extracted attn psum keep
            # the HAM clock at 8/8 through the AllReduce window so the tail
            # matmuls start warm. Results are never read (WAR on attn_pa gates
            # the start until after the extraction copies).
            for _ in range(40):
                nc.tensor.matmul(attn_pa, dwdiag[:, 0, :], va[:, 0:C],
                                 start=True, stop=True, skip_group_check=True)

            # ---------- phase 7: softmax (fq/fk already computed) ----------
            attn_f = smp.tile([HC, HEADS, HC], dt.float32)
            nc.scalar.dma_start(attn_f[:], cout[0:HC, :].rearrange("p (h c) -> p h c", h=HEADS))
            nc.vector.tensor_tensor(attn_f[:], attn_f[:],
                                    fq[:, :, None].to_broadcast((HC, HEADS, HC)), Alu.mult)
            nc.vector.tensor_tensor(attn_f[:], attn_f[:],
                                    fk32[:].rearrange("p (h c) -> p h c", h=HEADS), Alu.mult)
            ex = smp.tile([HC, HEADS, HC], dt.float32)
            nc.scalar.activation(ex[:], attn_f[:], Act.Exp)
            sm = smp.tile([HC, HEADS], dt.float32)
            nc.vector.tensor_reduce(sm[:], ex[:], mybir.AxisListType.X, Alu.add)
            nc.vector.reciprocal(sm[:], sm[:])
            nc.vector.tensor_tensor(ex[:], ex[:],
                                    sm[:, :, None].to_broadcast((HC, HEADS, HC)), Alu.mult)
            attn_bf = smp.tile([HC, HEADS, HC], dt.bfloat16)
            nc.vector.tensor_copy(attn_bf[:], ex[:])
            # block-diagonal NON-transposed attn: bd[i, j] = attn[i, j]
            bd_a = smp.tile([CA, CA], dt.bfloat16)
            bd_b = smp.tile([CB, CB], dt.bfloat16)
            nc.vector.memset(bd_a[:], 0.0)
            nc.vector.memset(bd_b[:], 0.0)
            qengs = (nc.sync, nc.gpsimd, nc.scalar)
            for h in range(HEADS):
                if h < 4:
                    qengs[h % 3].dma_start(bd_a[HC * h:HC * (h + 1), HC * h:HC * (h + 1)],
                                           attn_bf[:, h, :])
                else:
                    j = h - 4
                    qengs[h % 3].dma_start(bd_b[HC * j:HC * (j + 1), HC * j:HC * (j + 1)],
                                           attn_bf[:, h, :])

            # ---------- phase 8: fuse Wc^T = (proj @ attn_bd)^T = A^T-free form
            # WcT[j, o] = sum_i bd[i, j] * projT[i, o]  (j = v channel)
            pw1 = psp.tile([CA, C], dt.float32, tag="mm")
            pw2 = psp.tile([CB, C], dt.float32, tag="mm")
            nc.tensor.matmul(pw1[:], bd_a[:], projw_a[:, 0:C], start=True, stop=True)
            nc.tensor.matmul(pw2[:], bd_b[:], projw_b[0:CB, 0:C], start=True, stop=True)
            wcT_a = smp.tile([CA, C], dt.bfloat16)
            wcT_b = smp.tile([CA, C], dt.bfloat16)  # rows 64:128 zero (cancel k-upper)
            nc.vector.memset(wcT_b[:], 0.0)
            nc.scalar.copy(wcT_a[:], pw1[:])
            nc.scalar.copy(wcT_b[0:CB, :], pw2[:])

            # ---------- phase 9: out = Wc @ v, + residual ----------
            for i in range(NCH):
                s0 = 512 * i
                ppa = psp.tile([CA, 512], dt.float32, tag="mm")
                ppb = psp.tile([CB, 512], dt.float32, tag="mm")
                nc.tensor.matmul(ppa[:], wcT_a[:, 0:CA], va[:, s0:s0 + 512], start=True, stop=False)
                # rhs rows 64:128 hold dw'd k-upper; wcT_b zero rows cancel them
                nc.tensor.matmul(ppa[:], wcT_b[:, 0:CA], kvb_out[:, s0:s0 + 512], start=False, stop=True)
                nc.tensor.matmul(ppb[:], wcT_a[:, CA:C], va[:, s0:s0 + 512], start=True, stop=False)
                nc.tensor.matmul(ppb[:], wcT_b[:, CA:C], kvb_out[:, s0:s0 + 512], start=False, stop=True)
                xca_v = xca_s[:, s0:s0 + 512]
                if i < 8:
                    xcb_v = xcb_s[0:CB, s0:s0 + 512]
                else:
                    xcb_v = xcb_s[CB:CA, s0 - 4096:s0 - 4096 + 512]
                nc.vector.scalar_tensor_tensor(xca_v, ppa[:], -1.0, xca_v, Alu.mult, Alu.add)
                nc.vector.scalar_tensor_tensor(xcb_v, ppb[:], -1.0, xcb_v, Alu.mult, Alu.add)
                nc.sync.dma_start(out_t.ap()[0:CA, s0:s0 + 512], xca_v)
                nc.gpsimd.dma_start(out_t.ap()[CA:C, s0:s0 + 512], xcb_v)

    nc.compile()
    return nc


def _host_prep(inputs):
    x = np.asarray(inputs["x"], dtype=np.float32)
    y = np.asarray(inputs["y"], dtype=np.float32)
    kv_w = np.asarray(inputs["kv_w"], dtype=np.float32)[:, :, 0, 0]
    kv_dw = np.asarray(inputs["kv_dw_w"], dtype=np.float32)[:, 0]
    q_w = np.asarray(inputs["q_w"], dtype=np.float32)[:, :, 0, 0]
    q_dw = np.asarray(inputs["q_dw_w"], dtype=np.float32)
    proj_w = np.asarray(inputs["proj_w"], dtype=np.float32)[:, :, 0, 0]
    temp = np.asarray(inputs["temperature"], dtype=np.float32)[:, 0, 0]

    def kpad(a):  # [192, M] -> [256, M] with zero rows
        return np.concatenate([a, np.zeros((CP - C, a.shape[1]), a.dtype)], 0)

    f8 = ml_dtypes.float8_e4m3
    S_KV = 64.0

    def to8(a):
        return np.ascontiguousarray(np.clip(a, -224, 224).astype(f8))

    # kv output-channel permutation: [k 0:128 | v 128:192 ; k 128:192 | v 0:128]
    perm = np.concatenate([np.arange(0, 128), np.arange(320, 384),
                           np.arange(128, 192), np.arange(192, 320)])
    # fp8 DoubleRow weights [K=128, plane, out]: plane0 = in-ch 0:128,
    # plane1 = in-ch 128:192 (+zero rows). Scaled by S_KV; k rows scale-cancel
    # in the l2 norm, v rows are unscaled by folding 1/S_KV into proj_w.
    kvwp = kv_w[perm].T * S_KV          # [192, 384]
    kvw8 = np.zeros((128, 2, 2 * C), np.float32)
    kvw8[:, 0, :] = kvwp[0:128]
    kvw8[0:64, 1, :] = kvwp[128:192]
    kvw8 = to8(kvw8)
    # fold q_w (1x1) into the full 3x3: W2[o,i,ky,kx] = sum_c q_dw[o,c,..] q_w[c,i]
    W2 = np.einsum('ocyx,ci->oiyx', q_dw, q_w)
    # per-output-channel scaling (cancels exactly in the q l2 norm)
    sq = 120.0 / np.abs(W2).max(axis=(1, 2, 3))
    W2s = W2 * sq[:, None, None, None]
    # 14 K-planes matching y_f8: 0-8 = in-ch 0:128 tap t; 9+s = in-ch 128:192
    # [tap 2s ; tap 2s+1]
    qdw8 = np.zeros((128, 14, C), np.float32)
    for t in range(9):
        ky, kx = t // 3, t % 3
        qdw8[:, t, :] = W2s[:, 0:128, ky, kx].T
    for s in range(5):
        t0, t1 = 2 * s, 2 * s + 1
        qdw8[0:64, 9 + s, :] = W2s[:, 128:192, t0 // 3, t0 % 3].T
        if t1 < 9:
            qdw8[64:128, 9 + s, :] = W2s[:, 128:192, t1 // 3, t1 % 3].T
    qdw8 = to8(qdw8)
    kdw = kv_dw[:C].reshape(C, 9)
    vdw = kv_dw[C:].reshape(C, 9)
    dw_all = np.ascontiguousarray(np.concatenate(
        [kdw[0:128], vdw[128:192], kdw[128:192], vdw[0:128]], 0))
    dw_diag = np.zeros((9, 128, 128), np.float32)
    dw_diag2 = np.zeros((9, 128, 128), np.float32)
    mixdw = np.concatenate([vdw[128:192], kdw[128:192]], 0)
    for t in range(9):
        np.fill_diagonal(dw_diag[t], vdw[0:128, t])
        np.fill_diagonal(dw_diag2[t], mixdw[:, t])
    dw_diag = np.ascontiguousarray(dw_diag).astype(bf16)
    dw_diag2 = np.ascontiguousarray(dw_diag2).astype(bf16)
    proj_wT = np.ascontiguousarray(kpad(proj_w.T / S_KV)).astype(bf16)
    temp2 = np.ascontiguousarray(np.broadcast_to(temp.reshape(1, HEADS), (HC, HEADS)))

    def padded(arr, b, s):
        r0 = HP * s
        p = np.zeros((C, PH, PW), np.float32)
        lo, hi = max(r0 - 1, 0), min(r0 + HP + 1, H)
        p[:, lo - r0 + 1:hi - r0 + 1, 1:W + 1] = arr[b, :, lo:hi, :]
        return p

    def shard_x8(arr, b, s):
        p = padded(arr, b, s)
        x8 = np.zeros((128, 2, 8592), np.float32)
        x8[:, 0, :S_PAD] = p[0:128].reshape(128, S_PAD)
        x8[0:64, 1, :S_PAD] = p[128:192].reshape(64, S_PAD)
        return to8(x8)

    def shard_y8(arr, b, s):
        p = padded(arr, b, s)
        y8 = np.zeros((128, 14, HP, W), np.float32)
        for t in range(9):
            ky, kx = t // 3, t % 3
            y8[:, t] = p[0:128, ky:ky + HP, kx:kx + W]
        for sp in range(5):
            t0, t1 = 2 * sp, 2 * sp + 1
            ky, kx = t0 // 3, t0 % 3
            y8[0:64, 9 + sp] = p[128:192, ky:ky + HP, kx:kx + W]
            if t1 < 9:
                ky, kx = t1 // 3, t1 % 3
                y8[64:128, 9 + sp] = p[128:192, ky:ky + HP, kx:kx + W]
        # chunk-major: [128, 16 chunks, 14 planes, 512]
        return to8(np.ascontiguousarray(
            y8.reshape(128, 14, S_IN // 512, 512).transpose(0, 2, 1, 3)))

    in_maps = []
    for core in range(NCORES):
        b, s = core // 2, core % 2
        r0 = HP * s
        in_maps.append({
            "x_f8": shard_x8(x, b, s),
            "y_f8": shard_y8(y, b, s),
            "x_ctr": np.ascontiguousarray(
                x[b, :, r0:r0 + HP, :].reshape(C, S_IN).astype(bf16)),
            "kv_wT": kvw8, "qdw_T": qdw8,
            "dw_all": dw_all, "dw_diag": dw_diag, "dw_diag2": dw_diag2,
            "proj_wT": proj_wT,
            "temp": temp2,
        })
    return in_maps


LAST_RESULT = None


def kernel(**inputs):
    global LAST_RESULT
    from concourse.bass_utils import run_bass_kernel_spmd

    if "nc" not in _cache:
        _cache["nc"] = _build()
    nc = _cache["nc"]
    in_maps = _host_prep(inputs)
    res = run_bass_kernel_spmd(nc, in_maps, core_ids=list(range(NCORES)))
    LAST_RESULT = res
    out = np.empty((B, C, H, W), np.float32)
    for core in range(NCORES):
        b, s = core // 2, core % 2
        out[b, :, HP * s:HP * (s + 1), :] = \
            res.results[core]["out"].reshape(C, HP, W).astype(np.float32)
    return out

